# revision 13
# baseline (speedup 1.0000x reference)
"""Bass/Trainium2 kernel for a fused GRU cell.

  r   = sigmoid(x @ W_ir.T + h @ W_hr.T + b_r)
  z   = sigmoid(x @ W_iz.T + h @ W_hz.T + b_z)
  g   = tanh  (x @ W_ih.T + (r*h) @ W_hh.T + b_h)
  h_t = (1-z)*h + z*g

Sharding: data-parallel over the batch (8192 -> 1024 rows per core on 8
NeuronCores), weights replicated, no collectives.

Everything on-device is computed in a transposed layout ([hidden, batch]
with hidden on SBUF partitions) so that
  - the per-h-tile bias is a per-partition scalar (free with activation),
  - weight tiles land as natural [K,M] stationary operands,
  - all DMAs are contiguous (host numpy does every transpose/reshape).

Precision plan (tolerance is 2e-2 relative; measured headroom ~2x):
  - r gate runs fp8 e4m3 x fp8 e4m3 in MatmulPerfMode.DoubleRow (2 K-tiles
    per instruction -> 2x PE throughput). Operands pre-scaled on host
    (x*16, W*32); the sigmoid activation descales via its scale operand.
    The sigmoid's 1/4 slope damps fp8 noise; r is further smoothed through
    the W_hh contraction.
  - z and h gates run bf16 x bf16 (full PE rate, half the DMA of f32r).
  - Accumulation is always fp32 PSUM; activations/blend run f32 on
    Scalar/DVE; h_t is written back as bf16 (host upcasts).
DMA issues are spread across the sync/scalar/gpsimd queues so the tensor
engine's first matmul can start ~7us in, and the output DMAs don't pile
up on one queue at the tail.
"""

import sys

for _p in ("/opt/trn_rl_repo", "/root/.axon_site/_ro/trn_rl_repo"):
    if _p not in sys.path:
        sys.path.append(_p)

import numpy as np

P = 128          # SBUF partitions
BC_MAX = 512     # fp32 moving-operand / PSUM-bank max free dim
N_CORES = 8
SX = 16.0        # fp8 activation scale
SW = 32.0        # fp8 weight scale
QT = 6           # bf16 weight k-tiles per DMA slab
PQ = 6           # fp8 weight k-tile PAIRS per DMA slab

_PROG_CACHE = {}


def build_program(Bc, IN, H):
    """Build the per-core SPMD Bass program (identical on all cores)."""
    from contextlib import ExitStack

    from concourse import bacc, bass, mybir, tile
    from concourse.dt import dt

    KI, KH, NT = IN // P, H // P, H // P
    NJ = KI + KH                 # contraction tiles per gate per h-tile
    NP = NJ // 2                 # fp8 DoubleRow k-tile pairs
    NQ8 = NP // PQ               # fp8 slabs per h-tile
    NQ = NJ // QT                # bf16 slabs per h-tile per gate
    BC = min(BC_MAX, Bc)
    NB = Bc // BC
    f32, bf16, f8 = dt.float32, dt.bfloat16, dt.float8e4
    SIG = mybir.ActivationFunctionType.Sigmoid
    TANH = mybir.ActivationFunctionType.Tanh
    DR = mybir.MatmulPerfMode.DoubleRow
    DESCALE = 1.0 / (SX * SW)

    nc = bacc.Bacc("TRN2", debug=False)
    xt8_d = nc.declare_dram_parameter("xt8", [P, KI, Bc], f8, False)
    hp8_d = nc.declare_dram_parameter("hp8", [P, KH, Bc], f8, False)
    xt16_d = nc.declare_dram_parameter("xt16", [P, KI, Bc], bf16, False)
    hp16_d = nc.declare_dram_parameter("hp16", [P, KH, Bc], bf16, False)
    wr_d = nc.declare_dram_parameter("wr", [NT, NQ8, P, PQ * 2 * P], f8, False)
    wz_d = nc.declare_dram_parameter("wz", [NT, NQ, P, QT * P], bf16, False)
    wh_d = nc.declare_dram_parameter("wh", [NT, NQ, P, QT * P], bf16, False)
    b_d = nc.declare_dram_parameter("bias", [P, NT * 3], f32, False)
    out_d = nc.declare_dram_parameter("out", [NT, P, Bc], bf16, True)

    with ExitStack() as ctx:
        tc = ctx.enter_context(tile.TileContext(nc))
        res = ctx.enter_context(tc.tile_pool(name="res", bufs=1))
        wp = ctx.enter_context(tc.tile_pool(name="wp", bufs=8))
        pp = ctx.enter_context(
            tc.tile_pool(name="pp", bufs=4, space=bass.MemorySpace.PSUM)
        )
        op = ctx.enter_context(tc.tile_pool(name="op", bufs=8))
        zp = ctx.enter_context(tc.tile_pool(name="zp", bufs=4))

        # one tile per k-tile: dependencies stay per-128-row-block, so the
        # first matmul only waits on its own pair of input DMAs
        xt8 = [res.tile([P, 2, Bc], f8, name=f"xt8_{j}", tag=f"xt8_{j}") for j in range(KI // 2)]
        hp8 = [res.tile([P, 2, Bc], f8, name=f"hp8_{t}", tag=f"hp8_{t}") for t in range(KH // 2)]
        xt16 = [res.tile([P, Bc], bf16, name=f"xt16_{j}", tag=f"xt16_{j}") for j in range(KI)]
        hp16 = [res.tile([P, Bc], bf16, name=f"hp16_{t}", tag=f"hp16_{t}") for t in range(KH)]
        rh = [res.tile([P, Bc], bf16, name=f"rh_{t}", tag=f"rh_{t}") for t in range(KH)]
        bias = res.tile([P, NT * 3], f32, tag="bias")

        # startup DMAs, spread across queues so the PE can start early and the
        # early r-phase isn't starved: only the fp8 operands + first slabs
        # move up front; the bf16 operands (needed >=5us/90us later) trickle
        # in during the r-phase loop below.
        #   gpsimd : h-tile 0's r-weight slabs, hp16[0], bias, hp8[4..7]
        #   sync   : xt8, then every other weight slab in consumption order
        #   scalar : hp8[0..3], then activations
        slab0 = []
        for q in range(NQ8):
            s = wp.tile([P, PQ, 2, P], f8, name=f"w0_{q}", tag="w")
            nc.gpsimd.dma_start(out=s[:], in_=wr_d[0, q])
            slab0.append(s)
        for j in range(KI // 2):
            for u in range(2):
                nc.sync.dma_start(
                    out=xt8[j][:, u, :], in_=xt8_d[:, 2 * j + u, :]
                )
        nc.gpsimd.dma_start(out=hp16[0][:], in_=hp16_d[:, 0, :])
        nc.gpsimd.dma_start(out=bias[:], in_=b_d[:])
        for t in range(KH // 2):
            eng = nc.scalar if t < KH // 4 else nc.gpsimd
            for u in range(2):
                eng.dma_start(out=hp8[t][:, u, :], in_=hp8_d[:, 2 * t + u, :])

        def gate8(ps, hti):
            # fp8 DoubleRow: each matmul consumes k-tile pair (2p, 2p+1)
            for q in range(NQ8):
                if hti == 0:
                    slab = slab0[q]
                else:
                    slab = wp.tile([P, PQ, 2, P], f8, tag="w")
                    nc.sync.dma_start(out=slab[:], in_=wr_d[hti, q])
                for pp_ in range(PQ):
                    p2 = q * PQ + pp_
                    movp = xt8[p2] if 2 * p2 < KI else hp8[p2 - KI // 2]
                    for bc in range(NB):
                        sl = slice(bc * BC, (bc + 1) * BC)
                        nc.tensor.matmul(
                            ps[:, sl],
                            slab[:, pp_, :, :],
                            movp[:, :, sl],
                            start=(p2 == 0),
                            stop=(p2 == NP - 1),
                            perf_mode=DR,
                            skip_group_check=True,
                        )

        def gate16(ps, w_d, hti, srch):
            for q in range(NQ):
                slab = wp.tile([P, QT * P], bf16, tag="w")
                nc.sync.dma_start(out=slab[:], in_=w_d[hti, q])
                for jj in range(QT):
                    j = q * QT + jj
                    lhs = slab[:, jj * P : (jj + 1) * P]
                    mov = xt16[j] if j < KI else srch[j - KI]
                    for bc in range(NB):
                        sl = slice(bc * BC, (bc + 1) * BC)
                        nc.tensor.matmul(
                            ps[:, sl],
                            lhs,
                            mov[:, sl],
                            start=(j == 0),
                            stop=(j == NJ - 1),
                            skip_group_check=True,
                        )

        # ---- phase R: r = sigmoid((gi_r + gh_r)/(SX*SW) + b_r); rh = r * h ----
        for hti in range(NT):
            # trickle in the bf16 operands one tile per iteration (gpsimd)
            if hti + 1 < KH:
                nc.gpsimd.dma_start(
                    out=hp16[hti + 1][:], in_=hp16_d[:, hti + 1, :]
                )
            if hti >= NT - KI:
                j = hti - (NT - KI)
                nc.gpsimd.dma_start(out=xt16[j][:], in_=xt16_d[:, j, :])
            ps = pp.tile([P, Bc], f32, tag="ps")
            gate8(ps, hti)
            for bc in range(NB):
                sl = slice(bc * BC, (bc + 1) * BC)
                nc.scalar.activation(
                    ps[:, sl], ps[:, sl], SIG,
                    bias=bias[:, hti * 3 : hti * 3 + 1], scale=DESCALE,
                )
                nc.vector.tensor_mul(rh[hti][:, sl], ps[:, sl], hp16[hti][:, sl])

        # ---- phase ZH: z, g, h_t = h + z*(g - h) ----
        for hti in range(NT - 1):
            psz = pp.tile([P, Bc], f32, tag="ps")
            gate16(psz, wz_d, hti, hp16)
            psh = pp.tile([P, Bc], f32, tag="ps")
            gate16(psh, wh_d, hti, rh)
            for bc in range(NB):
                sl = slice(bc * BC, (bc + 1) * BC)
                zs = zp.tile([P, BC], f32, tag="zs")
                # scalar engine lands z straight in SBUF (frees the psum rule)
                nc.scalar.activation(
                    zs[:], psz[:, sl], SIG, bias=bias[:, hti * 3 + 1 : hti * 3 + 2]
                )
                nc.scalar.activation(
                    psh[:, sl], psh[:, sl], TANH,
                    bias=bias[:, hti * 3 + 2 : hti * 3 + 3],
                )
                nc.vector.tensor_sub(psh[:, sl], psh[:, sl], hp16[hti][:, sl])
                nc.vector.tensor_mul(psh[:, sl], zs[:], psh[:, sl])
                o = op.tile([P, BC], bf16, tag="o")
                nc.vector.tensor_add(o[:], psh[:, sl], hp16[hti][:, sl])
                nc.gpsimd.dma_start(out=out_d[hti, :, sl], in_=o[:])

        # last h-tile: run each gate bc-OUTER (its own 512-wide psum group
        # completes halfway through), h-gate before z-gate, so the
        # tanh/sub/sig/blend chains all overlap the remaining matmuls and
        # only a short sig/mul/add chain is exposed after the final matmul.
        hti = NT - 1

        def gate16_last(ps, w_d, srch, post):
            slabs = []
            for q in range(NQ):
                slab = wp.tile([P, QT * P], bf16, name=f"wl{q}", tag="w")
                nc.sync.dma_start(out=slab[:], in_=w_d[hti, q])
                slabs.append(slab)
            for bc in range(NB):
                sl = slice(bc * BC, (bc + 1) * BC)
                for j in range(NJ):
                    lhs = slabs[j // QT][:, (j % QT) * P : (j % QT + 1) * P]
                    mov = xt16[j] if j < KI else srch[j - KI]
                    nc.tensor.matmul(
                        ps[:, sl], lhs, mov[:, sl],
                        start=(j == 0), stop=(j == NJ - 1),
                        skip_group_check=True,
                    )
                post(bc, sl)

        psh = pp.tile([P, Bc], f32, tag="ps")

        def post_h(bc, sl):
            nc.scalar.activation(
                psh[:, sl], psh[:, sl], TANH,
                bias=bias[:, hti * 3 + 2 : hti * 3 + 3],
            )
            nc.vector.tensor_sub(psh[:, sl], psh[:, sl], hp16[hti][:, sl])

        gate16_last(psh, wh_d, rh, post_h)

        psz = pp.tile([P, Bc], f32, tag="ps")
        o15 = op.tile([P, Bc], bf16, name="o15", tag="o15")
        BP = BC // 2

        def post_z(bc, _sl):
            for half in range(2):
                sl = slice(bc * BC + half * BP, bc * BC + (half + 1) * BP)
                zs = zp.tile([P, BP], f32, tag="zs")
                nc.scalar.activation(
                    zs[:], psz[:, sl], SIG,
                    bias=bias[:, hti * 3 + 1 : hti * 3 + 2],
                )
                nc.vector.tensor_mul(psh[:, sl], zs[:], psh[:, sl])
                nc.vector.tensor_add(o15[:, sl], psh[:, sl], hp16[hti][:, sl])

        gate16_last(psz, wz_d, hp16, post_z)
        # whole-tile output in 4 partition-split DMAs (2KB lines, parallel
        # rings on queues that are idle by now)
        oeng = (nc.gpsimd, nc.sync)
        for c in range(4):
            pr = slice(c * (P // 4), (c + 1) * (P // 4))
            oeng[c % 2].dma_start(out=out_d[hti, pr, :], in_=o15[pr, :])

    nc.compile()
    return nc


def _ktiles(Wi, Wh):
    """Stack [Wi-tiles; Wh-tiles] -> (NT, NJ, P, P) transposed k-tile blocks.

    cat[hti, j][p, m] = W[hti*P + m, j*P + p], i.e. each 128x128 stationary
    tile is W.T for that (k-tile, h-tile) block.
    """
    H, IN = Wi.shape
    KI, KH, NT = IN // P, H // P, H // P
    ti = Wi.reshape(NT, P, KI, P).transpose(0, 2, 3, 1)  # (NT, KI, p, m)
    th = Wh.reshape(NT, P, KH, P).transpose(0, 2, 3, 1)  # (NT, KH, p, m)
    return np.concatenate([ti, th], axis=1)              # (NT, NJ, p, m)


def _pack_w_fp8(Wi, Wh):
    """fp8 DoubleRow slabs: (NT, NQ8, P, PQ*2*P), scaled by SW."""
    import ml_dtypes

    cat = _ktiles(Wi, Wh) * SW
    NT, NJ = cat.shape[:2]
    NQ8 = NJ // 2 // PQ
    slab = (
        cat.reshape(NT, NQ8, PQ, 2, P, P)
        .transpose(0, 1, 4, 2, 3, 5)
        .reshape(NT, NQ8, P, PQ * 2 * P)
    )
    return np.ascontiguousarray(slab).astype(ml_dtypes.float8_e4m3)


def _pack_w_bf16(Wi, Wh):
    """bf16 slabs: (NT, NQ, P, QT*P)."""
    import ml_dtypes

    cat = _ktiles(Wi, Wh)
    NT, NJ = cat.shape[:2]
    NQ = NJ // QT
    slab = (
        cat.reshape(NT, NQ, QT, P, P)
        .transpose(0, 1, 3, 2, 4)
        .reshape(NT, NQ, P, QT * P)
    )
    return np.ascontiguousarray(slab).astype(ml_dtypes.bfloat16)


def _pack_acts(a):
    """(Bc, D) -> (P, D//P, Bc) with [p, t, b] = a[b, t*P + p]."""
    Bc, D = a.shape
    return np.ascontiguousarray(a.T.reshape(D // P, P, Bc).transpose(1, 0, 2))


def run(x_t, h_prev, W_ir, W_iz, W_ih, W_hr, W_hz, W_hh, b_r, b_z, b_h,
        trace=False):
    import ml_dtypes
    from concourse.bass_utils import run_bass_kernel_spmd

    x_t = np.asarray(x_t, dtype=np.float32)
    h_prev = np.asarray(h_prev, dtype=np.float32)
    B, IN = x_t.shape
    H = h_prev.shape[1]
    assert B % N_CORES == 0
    Bc = B // N_CORES
    NT = H // P

    key = (Bc, IN, H)
    if key not in _PROG_CACHE:
        _PROG_CACHE[key] = build_program(Bc, IN, H)
    nc = _PROG_CACHE[key]

    wr = _pack_w_fp8(np.asarray(W_ir, np.float32), np.asarray(W_hr, np.float32))
    wz = _pack_w_bf16(np.asarray(W_iz, np.float32), np.asarray(W_hz, np.float32))
    wh = _pack_w_bf16(np.asarray(W_ih, np.float32), np.asarray(W_hh, np.float32))
    bias = np.ascontiguousarray(
        np.stack(
            [np.asarray(b_r, np.float32), np.asarray(b_z, np.float32),
             np.asarray(b_h, np.float32)], axis=-1
        ).reshape(NT, P, 3).transpose(1, 0, 2).reshape(P, NT * 3)
    )

    in_maps = []
    for c in range(N_CORES):
        rows = slice(c * Bc, (c + 1) * Bc)
        xp = _pack_acts(x_t[rows])
        hp = _pack_acts(h_prev[rows])
        in_maps.append({
            "xt8": (xp * SX).astype(ml_dtypes.float8_e4m3),
            "hp8": (hp * SX).astype(ml_dtypes.float8_e4m3),
            "xt16": xp.astype(ml_dtypes.bfloat16),
            "hp16": hp.astype(ml_dtypes.bfloat16),
            "wr": wr, "wz": wz, "wh": wh, "bias": bias,
        })

    kw = {}
    if trace:
        kw = dict(trace=True, trace_cores=[0])
    res = run_bass_kernel_spmd(nc, in_maps, core_ids=list(range(N_CORES)), **kw)

    outs = []
    for c in range(N_CORES):
        o = np.asarray(res.results[c]["out"], dtype=np.float32)  # (NT, P, Bc)
        outs.append(o.reshape(H, Bc).T)                          # (Bc, H)
    full = np.concatenate(outs, axis=0).astype(np.float32)
    return (full, res) if trace else full


def kernel(**inputs):
    return run(**inputs)
